# revision 7
# baseline (speedup 1.0000x reference)
"""Trainium2 Bass kernel for nn_DHSFNN_6038724018534 (v2, chunk-pipelined).

Two-layer dendritic-branch spiking net, data parallel over batch (8 cores x 32).

v2 structure: everything is chunked by 25 time steps and pipelined:
  cast:   x fp32 -> xbf2 bf16 DRAM, rows reordered (chunk, batch, t_local)
          so each chunk's x^T transpose is one contiguous [800,128] xbar DMA.
  chunk k: transpose-DMA -> mm1 (PE) -> c1 bias (ScalarE) -> dendrite scan
          (Pool engine) -> branch-sum adds (VectorE) -> V ring (3 chunks).
  The serial membrane/spike scan (msteps, VectorE) for chunk k-1 runs while
  chunk k's drive is produced; layer 2 (lagged 25 steps) and the softmax
  readout are also per-chunk, so PE/Pool/ScalarE/DVE all overlap.
"""

import numpy as np
import ml_dtypes

BF16 = ml_dtypes.bfloat16

B_FULL, T, IN, H, OUT, BR = 256, 250, 700, 256, 20, 4
NCORES = 8
B = B_FULL // NCORES          # 32 per core
ROWS = B * T                  # 8000
INP = 768                     # padded contraction dim
KCH = 6                       # k-chunks for matmul1 (last has 60 rows)
HR = H * BR                   # 1024
MCH = HR // 128               # 8
TC = 25                       # chunk length
NCHUNK = T // TC              # 10
LAG = 2 * TC
TAU = T + LAG                 # 275 merged-scan steps
VR = 3 * TC                   # V ring length (75)
CB = B * TC                   # 800 columns per chunk
CBB = B * (TC + 1)            # 832 with boundary col


def _sigmoid(v):
    return 1.0 / (1.0 + np.exp(-v.astype(np.float64)))


def _prep_constants(W1, b1, tau_n1, tau_m1, W2, b2, tau_n2, tau_m2, Wr, br, tau_mr):
    beta1 = _sigmoid(tau_n1)          # [H, BR]
    a1 = _sigmoid(tau_m1)             # [H]
    beta2 = _sigmoid(tau_n2)
    a2 = _sigmoid(tau_m2)
    ar = _sigmoid(tau_mr)             # [OUT]

    mj, mh = np.divmod(np.arange(HR), H)          # m = j*H + h
    orig = mh * BR + mj
    scale1 = (1.0 - beta1[mh, mj]) * (1.0 - a1[mh])
    scale2 = (1.0 - beta2[mh, mj]) * (1.0 - a2[mh])

    w1p = W1.astype(np.float64)[orig] * scale1[:, None]
    w2p = W2.astype(np.float64)[orig] * scale2[:, None]
    b1p = b1.astype(np.float64)[orig] * scale1
    b2p = b2.astype(np.float64)[orig] * scale2

    w1pad = np.zeros((HR, INP), np.float64)
    w1pad[:, :IN] = w1p
    w1t = np.ascontiguousarray(
        w1pad.T.reshape(KCH, 128, HR).transpose(1, 0, 2).reshape(128, KCH * HR)
    ).astype(BF16)
    w2t = np.ascontiguousarray(
        w2p.T.reshape(2, 128, HR).transpose(1, 0, 2).reshape(128, 2 * HR)
    ).astype(BF16)

    b1c = np.ascontiguousarray(b1p.reshape(MCH, 128).T).astype(np.float32)
    b2c = np.ascontiguousarray(b2p.reshape(MCH, 128).T).astype(np.float32)

    beta1_m = beta1[mh, mj].reshape(MCH, 128).T          # [128, MCH]
    beta2_m = beta2[mh, mj].reshape(MCH, 128).T

    def bz(beta_m):
        # [128, MCH, B*(TC+1)] with 0 at each (b, i==0) boundary col
        z = np.repeat(beta_m[:, :, None], B * (TC + 1), axis=2).reshape(
            128, MCH, B, TC + 1)
        z[:, :, :, 0] = 0.0
        return z.reshape(128, MCH, CBB).astype(BF16)

    bz1c = bz(beta1_m)
    bz2 = bz(beta2_m)

    abc = np.empty((128, 2, 2, B), np.float64)
    for hh in range(2):
        abc[:, 0, hh, :] = a1[hh * 128:(hh + 1) * 128, None]
        abc[:, 1, hh, :] = a2[hh * 128:(hh + 1) * 128, None]
    abc = abc.astype(BF16)

    wrp = Wr.astype(np.float64) * (1.0 - ar)[:, None]
    wrt = np.ascontiguousarray(
        wrp.T.reshape(2, 128, OUT).transpose(1, 0, 2).reshape(128, 2 * OUT)
    ).astype(BF16)
    brc = (br.astype(np.float64) * (1.0 - ar)).astype(np.float32).reshape(OUT, 1)

    # mr-scan coefficients: [20, B*(TC+1)], 0 at boundary cols
    bzr = np.repeat(ar[:, None], B * (TC + 1), axis=1).reshape(OUT, B, TC + 1)
    bzr[:, :, 0] = 0.0
    bzr = bzr.reshape(OUT, CBB).astype(np.float32)

    # esel8[p, l*B + b] = ((p + 4*l) % 32 == b), for 100-col softmax blocks
    esel8 = np.zeros((100, 9, B), np.float32)
    for l in range(8):
        for p in range(100):
            esel8[p, l, (p + 4 * l) % B] = 1.0
    # slot 8: like l=1 but rows p<28 zeroed (warmup mask for chunk 0, block 1)
    esel8[28:, 8, :] = esel8[28:, 1, :]
    esel8 = esel8.reshape(100, 9 * B)

    idm = np.eye(OUT, dtype=np.float32)

    return dict(w1t=w1t, w2t=w2t, b1c=b1c, b2c=b2c, bz1c=bz1c, bz2=bz2,
                abc=abc, wrt=wrt, brc=brc, bzr=bzr, esel8=esel8, idm=idm)


def _build_program(num_devices=NCORES):
    from contextlib import ExitStack
    import concourse.bacc as bacc
    import concourse.tile as tile
    from concourse import mybir

    dt = mybir.dt
    Alu = mybir.AluOpType
    Act = mybir.ActivationFunctionType

    nc = bacc.Bacc("TRN2", target_bir_lowering=False, debug=False,
                   num_devices=num_devices)

    x = nc.dram_tensor("x", [ROWS, IN], dt.float32, kind="ExternalInput").ap()
    cons = {}
    for name, shape, d in [
        ("w1t", [128, KCH * HR], dt.bfloat16),
        ("w2t", [128, 2 * HR], dt.bfloat16),
        ("b1c", [128, MCH], dt.float32),
        ("b2c", [128, MCH], dt.float32),
        ("bz1c", [128, MCH, CBB], dt.bfloat16),
        ("bz2", [128, MCH, CBB], dt.bfloat16),
        ("abc", [128, 2, 2, B], dt.bfloat16),
        ("wrt", [128, 2 * OUT], dt.bfloat16),
        ("brc", [OUT, 1], dt.float32),
        ("bzr", [OUT, CBB], dt.float32),
        ("esel8", [100, 9 * B], dt.float32),
        ("idm", [OUT, OUT], dt.float32),
    ]:
        cons[name] = nc.dram_tensor(name, shape, d, kind="ExternalInput").ap()
    out = nc.dram_tensor("outv", [B, OUT], dt.float32, kind="ExternalOutput").ap()

    xbf2 = nc.dram_tensor("xbf2", [NCHUNK * CB, INP], dt.bfloat16).ap()

    with tile.TileContext(nc) as tc, ExitStack() as ctx:
        cpool = ctx.enter_context(tc.tile_pool(name="consts", bufs=1))
        sb = {}
        for name in cons:
            ap = cons[name]
            t = cpool.tile(list(ap.shape), ap.dtype, name=f"sb_{name}",
                           tag=f"sb_{name}")
            nc.gpsimd.dma_start(out=t[:], in_=ap[:])
            sb[name] = t

        gpool = ctx.enter_context(tc.tile_pool(name="glob", bufs=1))
        V = gpool.tile([128, 2, 2, B, VR], dt.bfloat16)      # drive ring
        S = gpool.tile([128, 2, 2, B, 2 * TC], dt.bfloat16)  # spike ring
        M = gpool.tile([128, 2, 2, B], dt.bfloat16)
        ZC = gpool.tile([128, 2, 2, B], dt.bfloat16)
        D1K = gpool.tile([128, MCH, B, 1], dt.bfloat16)
        D2K = gpool.tile([128, MCH, B, 1], dt.bfloat16)
        MRK = gpool.tile([OUT, B, 1], dt.float32)

        nc.vector.memset(M[:], 0.0)
        nc.vector.memset(ZC[:], 0.0)
        nc.vector.memset(D1K[:], 0.0)
        nc.vector.memset(D2K[:], 0.0)
        nc.vector.memset(MRK[:], 0.0)
        nc.vector.memset(V[:, 1, :, :, 0:2 * TC], 0.0)  # layer-2 drive, tau<50

        # ---------- cast: x -> xbf2, rows reordered (k, b, tl) ----------
        castp = ctx.enter_context(tc.tile_pool(name="cast", bufs=2))
        xv = x.rearrange("(b t) c -> b t c", t=T)
        # out view: [k, b, tl, c] row = k*800 + b*25 + tl
        ov = xbf2.rearrange("(k b tl) c -> k b tl c", b=B, tl=TC)
        for h in range(2):
            for g in range(B // 2):
                tf = castp.tile([125, 2, IN], dt.float32, tag="castf")
                nc.sync.dma_start(
                    out=tf[:],
                    in_=xv[2 * g:2 * g + 2, 125 * h:125 * (h + 1), :]
                        .rearrange("b t c -> t b c"),
                )
                tb = castp.tile([125, 2, IN], dt.bfloat16, tag="castb")
                nc.scalar.copy(tb[:], tf[:])
                # SBUF [125(t), 2(b), c] -> DRAM, one DMA per chunk row
                for kl in range(5):
                    nc.sync.dma_start(
                        out=ov[5 * h + kl, 2 * g:2 * g + 2, :, 0:IN]
                            .rearrange("b tl c -> tl b c"),
                        in_=tb[kl * TC:(kl + 1) * TC, :, :],
                    )

        # ---------- pools for the chunk pipeline ----------
        xtp = ctx.enter_context(tc.tile_pool(name="xt", bufs=13))
        c1p = ctx.enter_context(tc.tile_pool(name="c1", bufs=3))
        c2p = ctx.enter_context(tc.tile_pool(name="c2", bufs=3))
        d1p = ctx.enter_context(tc.tile_pool(name="d1", bufs=10))
        d2p = ctx.enter_context(tc.tile_pool(name="d2", bufs=10))
        dap = ctx.enter_context(tc.tile_pool(name="da", bufs=4))
        rop = ctx.enter_context(tc.tile_pool(name="ro", bufs=2))
        smp = ctx.enter_context(tc.tile_pool(name="sm", bufs=4))
        psAB = ctx.enter_context(tc.tile_pool(name="psAB", bufs=2, space="PSUM"))
        ps3p = ctx.enter_context(tc.tile_pool(name="ps3", bufs=1, space="PSUM"))
        psTp = ctx.enter_context(tc.tile_pool(name="psT", bufs=1, space="PSUM"))
        psXp = ctx.enter_context(tc.tile_pool(name="psX", bufs=1, space="PSUM"))

        psAcc = psXp.tile([B, OUT], dt.float32)
        psT = psTp.tile([100, 2, OUT], dt.float32)

        xts = {}

        def phaseA_dma(k):
            lst = []
            for kc in range(KCH):
                xt = xtp.tile([128, CB], dt.bfloat16, tag="xt")
                nc.sync.dma_start(
                    out=xt[:],
                    in_=xbf2[k * CB:(k + 1) * CB, kc * 128:(kc + 1) * 128],
                    transpose=True,
                )
                lst.append(xt)
            xts[k] = lst

        def phaseA_compute(k):
            xt = xts.pop(k)
            d1s = []
            for mc in range(MCH):
                ps = psAB.tile([128, 1024], dt.float32, tag="psAB", name="psAB")
                for kc in range(KCH):
                    kp = 128 if kc < KCH - 1 else IN - 128 * (KCH - 1)
                    lhsT = sb["w1t"][0:kp, kc * HR + mc * 128:
                                     kc * HR + (mc + 1) * 128]
                    for nt in range(2):
                        nc.tensor.matmul(
                            ps[:, nt * 512:nt * 512 + 400], lhsT,
                            xt[kc][0:kp, nt * 400:(nt + 1) * 400],
                            start=(kc == 0), stop=(kc == KCH - 1),
                        )
                c1 = c1p.tile([128, B, TC + 1], dt.bfloat16, tag="c1")
                nc.scalar.copy(c1[:, :, 0:1], D1K[:, mc, :, :])
                nc.scalar.activation(
                    c1[:].rearrange("p (nt b) i -> p nt b i", nt=2)[:, :, :, 1:TC + 1],
                    ps[:].rearrange("p (nt x) -> p nt x", nt=2)[:, :, 0:400]
                        .rearrange("p nt (b t) -> p nt b t", t=TC),
                    Act.Identity, bias=sb["b1c"][:, mc:mc + 1], scale=1.0,
                )
                d1 = d1p.tile([128, B, TC + 1], dt.bfloat16, tag="d1")
                nc.vector.tensor_tensor_scan(
                    d1[:].rearrange("p b t -> p (b t)"),
                    sb["bz1c"][:, mc, :],
                    c1[:].rearrange("p b t -> p (b t)"),
                    0.0, op0=Alu.mult, op1=Alu.add,
                )
                nc.scalar.copy(D1K[:, mc, :, :], d1[:, :, TC:TC + 1])
                d1s.append(d1)
            slot = (k * TC) % VR
            for hh in range(2):
                ta = dap.tile([128, B, TC], dt.bfloat16, tag="ta")
                tb2 = dap.tile([128, B, TC], dt.bfloat16, tag="tb")
                nc.gpsimd.tensor_tensor(
                    ta[:], d1s[hh][:, :, 1:], d1s[2 + hh][:, :, 1:], Alu.add)
                nc.gpsimd.tensor_tensor(
                    tb2[:], d1s[4 + hh][:, :, 1:], d1s[6 + hh][:, :, 1:], Alu.add)
                nc.gpsimd.tensor_tensor(
                    V[:, 0, hh, :, slot:slot + TC], ta[:], tb2[:], Alu.add)

        def phaseA_zero(k):
            slot = (k * TC) % VR
            nc.vector.memset(V[:, 0, :, :, slot:slot + TC], 0.0)

        def msteps(kk):
            for tau in range(kk * TC, (kk + 1) * TC):
                slot = tau % (2 * TC)
                sprev = (ZC[:] if tau == 0
                         else S[:, :, :, :, (tau - 1) % (2 * TC)])
                n = dap.tile([128, 2, 2, B], dt.bfloat16, tag="n")
                nc.vector.tensor_tensor(n[:], M[:], sprev, Alu.subtract)
                g = dap.tile([128, 2, 2, B], dt.bfloat16, tag="g")
                nc.vector.tensor_tensor(g[:], n[:], sb["abc"][:], Alu.mult)
                nc.vector.tensor_tensor(M[:], g[:], V[:, :, :, :, tau % VR],
                                        Alu.add)
                nc.vector.tensor_scalar(
                    S[:, :, :, :, slot], M[:], 1.0, None, op0=Alu.is_gt)

        def layer2(kk):
            base = (kk * TC) % (2 * TC)
            d2s = []
            for mc in range(MCH):
                ps = psAB.tile([128, 1024], dt.float32, tag="psAB", name="psAB")
                for kc in range(2):
                    for nt in range(2):
                        rhs = S[:, 0, kc, nt * 16:(nt + 1) * 16, base:base + TC]
                        nc.tensor.matmul(
                            ps[:, nt * 512:nt * 512 + 400],
                            sb["w2t"][:, kc * HR + mc * 128:
                                      kc * HR + (mc + 1) * 128],
                            rhs, start=(kc == 0), stop=(kc == 1),
                        )
                c2 = c2p.tile([128, B, TC + 1], dt.bfloat16, tag="c2")
                nc.scalar.copy(c2[:, :, 0:1], D2K[:, mc, :, :])
                nc.scalar.activation(
                    c2[:].rearrange("p (nt b) i -> p nt b i", nt=2)[:, :, :, 1:TC + 1],
                    ps[:].rearrange("p (nt x) -> p nt x", nt=2)[:, :, 0:400]
                        .rearrange("p nt (b t) -> p nt b t", t=TC),
                    Act.Identity, bias=sb["b2c"][:, mc:mc + 1], scale=1.0,
                )
                d2 = d2p.tile([128, B, TC + 1], dt.bfloat16, tag="d2")
                nc.vector.tensor_tensor_scan(
                    d2[:].rearrange("p b t -> p (b t)"),
                    sb["bz2"][:, mc, :],
                    c2[:].rearrange("p b t -> p (b t)"),
                    0.0, op0=Alu.mult, op1=Alu.add,
                )
                nc.scalar.copy(D2K[:, mc, :, :], d2[:, :, TC:TC + 1])
                d2s.append(d2)
            slot = ((kk + 2) * TC) % VR
            for hh in range(2):
                ta = dap.tile([128, B, TC], dt.bfloat16, tag="ta")
                tb2 = dap.tile([128, B, TC], dt.bfloat16, tag="tb")
                nc.gpsimd.tensor_tensor(
                    ta[:], d2s[hh][:, :, 1:], d2s[2 + hh][:, :, 1:], Alu.add)
                nc.gpsimd.tensor_tensor(
                    tb2[:], d2s[4 + hh][:, :, 1:], d2s[6 + hh][:, :, 1:], Alu.add)
                nc.gpsimd.tensor_tensor(
                    V[:, 1, hh, :, slot:slot + TC], ta[:], tb2[:], Alu.add)

        def readout(kk):
            # output-time chunk kk; layer-2 spikes live at tau chunk kk+1
            base = ((kk + 2) * TC) % (2 * TC)
            ps3 = ps3p.tile([OUT, 1024], dt.float32, tag="ps3")
            for kc in range(2):
                for nt in range(2):
                    rhs = S[:, 1, kc, nt * 16:(nt + 1) * 16, base:base + TC]
                    nc.tensor.matmul(
                        ps3[:, nt * 512:nt * 512 + 400],
                        sb["wrt"][:, kc * OUT:(kc + 1) * OUT], rhs,
                        start=(kc == 0), stop=(kc == 1),
                    )
            c3 = rop.tile([OUT, B, TC + 1], dt.float32, tag="c3")
            nc.scalar.copy(c3[:, :, 0:1], MRK[:])
            nc.scalar.activation(
                c3[:].rearrange("p (nt b) i -> p nt b i", nt=2)[:, :, :, 1:TC + 1],
                ps3[:].rearrange("p (nt x) -> p nt x", nt=2)[:, :, 0:400]
                     .rearrange("p nt (b t) -> p nt b t", t=TC),
                Act.Identity, bias=sb["brc"][:], scale=1.0,
            )
            mrt = rop.tile([OUT, B, TC + 1], dt.float32, tag="mrt")
            nc.vector.tensor_tensor_scan(
                mrt[:].rearrange("p b t -> p (b t)"),
                sb["bzr"][:],
                c3[:].rearrange("p b t -> p (b t)"),
                0.0, op0=Alu.mult, op1=Alu.add,
            )
            nc.scalar.copy(MRK[:], mrt[:, :, TC:TC + 1])
            ex = rop.tile([OUT, TC, B], dt.float32, tag="ex")
            nc.scalar.activation(
                ex[:].rearrange("p t b -> p b t"),
                mrt[:, :, 1:TC + 1], Act.Exp,
            )
            # softmax + accumulate: 8 blocks of 100 t-major columns
            exf = ex[:].rearrange("p t b -> p (t b)")
            for l in range(8):
                pt = psT[:, l % 2, :]
                nc.tensor.transpose(pt, exf[:, l * 100:(l + 1) * 100],
                                    sb["idm"][:])
                rs = smp.tile([100, 1], dt.float32, tag="rs")
                nc.vector.tensor_reduce(rs[:], pt,
                                        axis=mybir.AxisListType.X, op=Alu.add)
                ri = smp.tile([100, 1], dt.float32, tag="ri")
                nc.vector.reciprocal(ri[:], rs[:])
                sm = smp.tile([100, OUT], dt.float32, tag="sm")
                nc.vector.tensor_scalar(sm[:], pt, ri[:], None, op0=Alu.mult)
                if kk == 0 and l == 0:
                    continue  # cols 0-99 are warmup (t<4)
                le = 8 if (kk == 0 and l == 1) else l
                nc.tensor.matmul(
                    psAcc[:],
                    sb["esel8"][:, le * B:(le + 1) * B],
                    sm[:],
                    start=(kk == 0 and l == 1),
                    stop=(kk == NCHUNK - 1 and l == 7),
                )

        # ---------- the pipelined chunk loop ----------
        for k in range(NCHUNK + 3):
            if k < NCHUNK:
                phaseA_dma(k)
            if k in (NCHUNK, NCHUNK + 1):
                phaseA_zero(k)
            if 1 <= k <= NCHUNK + 2:
                msteps(k - 1)
            if 1 <= k <= NCHUNK:
                layer2(k - 1)
            if k >= 3:
                readout(k - 3)
            if k < NCHUNK:
                phaseA_compute(k)

        accS = smp.tile([B, OUT], dt.float32, tag="acc")
        nc.scalar.copy(accS[:], psAcc[:])
        nc.scalar.dma_start(out=out[:], in_=accS[:])

    nc.compile()
    return nc


_NC_CACHE = {}


def _get_program(num_devices=NCORES):
    if num_devices not in _NC_CACHE:
        _NC_CACHE[num_devices] = _build_program(num_devices)
    return _NC_CACHE[num_devices]


def make_in_maps(x, consts):
    xs = np.ascontiguousarray(x.astype(np.float32).reshape(NCORES, ROWS, IN))
    return [{"x": xs[c], **consts} for c in range(NCORES)]


def kernel(x, W1, b1, tau_n1, tau_m1, W2, b2, tau_n2, tau_m2, Wr, br, tau_mr):
    from concourse.bass_utils import run_bass_kernel_spmd

    consts = _prep_constants(W1, b1, tau_n1, tau_m1, W2, b2, tau_n2, tau_m2,
                             Wr, br, tau_mr)
    nc = _get_program()
    in_maps = make_in_maps(np.asarray(x), consts)
    res = run_bass_kernel_spmd(nc, in_maps, list(range(NCORES)))
    outk = "outv"
    o = np.concatenate([res.results[c][outk] for c in range(NCORES)], axis=0)
    return o.astype(np.float32)


# revision 8
# speedup vs baseline: 2.2794x; 2.2794x over previous
"""Trainium2 Bass kernel for nn_DHSFNN_6038724018534 (v2, chunk-pipelined).

Two-layer dendritic-branch spiking net, data parallel over batch (8 cores x 32).

v2 structure: everything is chunked by 25 time steps and pipelined:
  cast:   x fp32 -> xbf2 bf16 DRAM, rows reordered (chunk, batch, t_local)
          so each chunk's x^T transpose is one contiguous [800,128] xbar DMA.
  chunk k: transpose-DMA -> mm1 (PE) -> c1 bias (ScalarE) -> dendrite scan
          (Pool engine) -> branch-sum adds (VectorE) -> V ring (3 chunks).
  The serial membrane/spike scan (msteps, VectorE) for chunk k-1 runs while
  chunk k's drive is produced; layer 2 (lagged 25 steps) and the softmax
  readout are also per-chunk, so PE/Pool/ScalarE/DVE all overlap.
"""

import numpy as np
import ml_dtypes

BF16 = ml_dtypes.bfloat16

B_FULL, T, IN, H, OUT, BR = 256, 250, 700, 256, 20, 4
NCORES = 8
B = B_FULL // NCORES          # 32 per core
ROWS = B * T                  # 8000
INP = 768                     # padded contraction dim
KCH = 6                       # k-chunks for matmul1 (last has 60 rows)
HR = H * BR                   # 1024
MCH = HR // 128               # 8
TC = 25                       # chunk length
NCHUNK = T // TC              # 10
LAG = 2 * TC
TAU = T + LAG                 # 275 merged-scan steps
VR = 3 * TC                   # V ring length (75)
CB = B * TC                   # 800 columns per chunk
CBB = B * (TC + 1)            # 832 with boundary col


def _sigmoid(v):
    return 1.0 / (1.0 + np.exp(-v.astype(np.float64)))


def _prep_constants(W1, b1, tau_n1, tau_m1, W2, b2, tau_n2, tau_m2, Wr, br, tau_mr):
    beta1 = _sigmoid(tau_n1)          # [H, BR]
    a1 = _sigmoid(tau_m1)             # [H]
    beta2 = _sigmoid(tau_n2)
    a2 = _sigmoid(tau_m2)
    ar = _sigmoid(tau_mr)             # [OUT]

    mj, mh = np.divmod(np.arange(HR), H)          # m = j*H + h
    orig = mh * BR + mj
    scale1 = (1.0 - beta1[mh, mj]) * (1.0 - a1[mh])
    scale2 = (1.0 - beta2[mh, mj]) * (1.0 - a2[mh])

    w1p = W1.astype(np.float64)[orig] * scale1[:, None]
    w2p = W2.astype(np.float64)[orig] * scale2[:, None]
    b1p = b1.astype(np.float64)[orig] * scale1
    b2p = b2.astype(np.float64)[orig] * scale2

    w1pad = np.zeros((HR, INP), np.float64)
    w1pad[:, :IN] = w1p
    w1t = np.ascontiguousarray(
        w1pad.T.reshape(KCH, 128, HR).transpose(1, 0, 2).reshape(128, KCH * HR)
    ).astype(BF16)
    w2t = np.ascontiguousarray(
        w2p.T.reshape(2, 128, HR).transpose(1, 0, 2).reshape(128, 2 * HR)
    ).astype(BF16)

    b1c = np.ascontiguousarray(b1p.reshape(MCH, 128).T).astype(np.float32)
    b2c = np.ascontiguousarray(b2p.reshape(MCH, 128).T).astype(np.float32)

    beta1_m = beta1[mh, mj].reshape(MCH, 128).T          # [128, MCH]
    beta2_m = beta2[mh, mj].reshape(MCH, 128).T

    def bz(beta_m):
        # [128, MCH, B*(TC+1)] with 0 at each (b, i==0) boundary col
        z = np.repeat(beta_m[:, :, None], B * (TC + 1), axis=2).reshape(
            128, MCH, B, TC + 1)
        z[:, :, :, 0] = 0.0
        return z.reshape(128, MCH, CBB).astype(BF16)

    bz1c = bz(beta1_m)
    bz2 = bz(beta2_m)

    abc = np.empty((128, 2, 2, B), np.float64)
    for hh in range(2):
        abc[:, 0, hh, :] = a1[hh * 128:(hh + 1) * 128, None]
        abc[:, 1, hh, :] = a2[hh * 128:(hh + 1) * 128, None]
    abc = abc.astype(BF16)

    wrp = Wr.astype(np.float64) * (1.0 - ar)[:, None]
    wrt = np.ascontiguousarray(
        wrp.T.reshape(2, 128, OUT).transpose(1, 0, 2).reshape(128, 2 * OUT)
    ).astype(BF16)
    brc = (br.astype(np.float64) * (1.0 - ar)).astype(np.float32).reshape(OUT, 1)

    # mr-scan coefficients: [20, B*(TC+1)], 0 at boundary cols
    bzr = np.repeat(ar[:, None], B * (TC + 1), axis=1).reshape(OUT, B, TC + 1)
    bzr[:, :, 0] = 0.0
    bzr = bzr.reshape(OUT, CBB).astype(np.float32)

    # esel8[p, l*B + b] = ((p + 4*l) % 32 == b), for 100-col softmax blocks
    esel8 = np.zeros((100, 9, B), np.float32)
    for l in range(8):
        for p in range(100):
            esel8[p, l, (p + 4 * l) % B] = 1.0
    # slot 8: like l=1 but rows p<28 zeroed (warmup mask for chunk 0, block 1)
    esel8[28:, 8, :] = esel8[28:, 1, :]
    esel8 = esel8.reshape(100, 9 * B)

    idm = np.eye(OUT, dtype=np.float32)

    return dict(w1t=w1t, w2t=w2t, b1c=b1c, b2c=b2c, bz1c=bz1c, bz2=bz2,
                abc=abc, wrt=wrt, brc=brc, bzr=bzr, esel8=esel8, idm=idm)


def _build_program(num_devices=NCORES):
    from contextlib import ExitStack
    import concourse.bacc as bacc
    import concourse.tile as tile
    from concourse import mybir

    dt = mybir.dt
    Alu = mybir.AluOpType
    Act = mybir.ActivationFunctionType

    nc = bacc.Bacc("TRN2", target_bir_lowering=False, debug=False,
                   num_devices=num_devices)

    x = nc.dram_tensor("x", [ROWS, IN], dt.float32, kind="ExternalInput").ap()
    cons = {}
    for name, shape, d in [
        ("w1t", [128, KCH * HR], dt.bfloat16),
        ("w2t", [128, 2 * HR], dt.bfloat16),
        ("b1c", [128, MCH], dt.float32),
        ("b2c", [128, MCH], dt.float32),
        ("bz1c", [128, MCH, CBB], dt.bfloat16),
        ("bz2", [128, MCH, CBB], dt.bfloat16),
        ("abc", [128, 2, 2, B], dt.bfloat16),
        ("wrt", [128, 2 * OUT], dt.bfloat16),
        ("brc", [OUT, 1], dt.float32),
        ("bzr", [OUT, CBB], dt.float32),
        ("esel8", [100, 9 * B], dt.float32),
        ("idm", [OUT, OUT], dt.float32),
    ]:
        cons[name] = nc.dram_tensor(name, shape, d, kind="ExternalInput").ap()
    out = nc.dram_tensor("outv", [B, OUT], dt.float32, kind="ExternalOutput").ap()

    xbf2 = nc.dram_tensor("xbf2", [NCHUNK * CB, INP], dt.bfloat16).ap()

    with tile.TileContext(nc) as tc, ExitStack() as ctx:
        cpool = ctx.enter_context(tc.tile_pool(name="consts", bufs=1))
        sb = {}
        for name in cons:
            ap = cons[name]
            t = cpool.tile(list(ap.shape), ap.dtype, name=f"sb_{name}",
                           tag=f"sb_{name}")
            nc.gpsimd.dma_start(out=t[:], in_=ap[:])
            sb[name] = t

        gpool = ctx.enter_context(tc.tile_pool(name="glob", bufs=1))
        V = gpool.tile([128, 2, 2, B, VR], dt.bfloat16)      # drive ring
        S = gpool.tile([128, 2, 2, B, 2 * TC], dt.bfloat16)  # spike ring
        M = gpool.tile([128, 2, 2, B], dt.bfloat16)
        ZC = gpool.tile([128, 2, 2, B], dt.bfloat16)
        D1K = gpool.tile([128, MCH, B, 1], dt.bfloat16)
        D2K = gpool.tile([128, MCH, B, 1], dt.bfloat16)
        MRK = gpool.tile([OUT, B, 1], dt.float32)

        nc.vector.memset(M[:], 0.0)
        nc.vector.memset(ZC[:], 0.0)
        nc.vector.memset(D1K[:], 0.0)
        nc.vector.memset(D2K[:], 0.0)
        nc.vector.memset(MRK[:], 0.0)
        nc.vector.memset(V[:, 1, :, :, 0:2 * TC], 0.0)  # layer-2 drive, tau<50

        # ---------- cast: x -> xbf2, rows reordered (k, b, tl) ----------
        castp = ctx.enter_context(tc.tile_pool(name="cast", bufs=2))
        xv = x.rearrange("(b t) c -> b t c", t=T)
        # out view: [k, b, tl, c] row = k*800 + b*25 + tl
        ov = xbf2.rearrange("(k b tl) c -> k b tl c", b=B, tl=TC)
        for h in range(2):
            for g in range(B // 2):
                tf = castp.tile([125, 2, IN], dt.float32, tag="castf")
                nc.sync.dma_start(
                    out=tf[:],
                    in_=xv[2 * g:2 * g + 2, 125 * h:125 * (h + 1), :]
                        .rearrange("b t c -> t b c"),
                )
                tb = castp.tile([125, 2, IN], dt.bfloat16, tag="castb")
                nc.scalar.copy(tb[:], tf[:])
                # SBUF [125(t), 2(b), c] -> DRAM, one DMA per chunk row
                for kl in range(5):
                    nc.sync.dma_start(
                        out=ov[5 * h + kl, 2 * g:2 * g + 2, :, 0:IN]
                            .rearrange("b tl c -> tl b c"),
                        in_=tb[kl * TC:(kl + 1) * TC, :, :],
                    )

        # ---------- pools for the chunk pipeline ----------
        xtp = ctx.enter_context(tc.tile_pool(name="xt", bufs=13))
        c1p = ctx.enter_context(tc.tile_pool(name="c1", bufs=3))
        c2p = ctx.enter_context(tc.tile_pool(name="c2", bufs=3))
        d1p = ctx.enter_context(tc.tile_pool(name="d1", bufs=10))
        d2p = ctx.enter_context(tc.tile_pool(name="d2", bufs=10))
        dap = ctx.enter_context(tc.tile_pool(name="da", bufs=4))
        rop = ctx.enter_context(tc.tile_pool(name="ro", bufs=2))
        smp = ctx.enter_context(tc.tile_pool(name="sm", bufs=4))
        psAB = ctx.enter_context(tc.tile_pool(name="psAB", bufs=2, space="PSUM"))
        ps3p = ctx.enter_context(tc.tile_pool(name="ps3", bufs=1, space="PSUM"))
        psTp = ctx.enter_context(tc.tile_pool(name="psT", bufs=1, space="PSUM"))
        psXp = ctx.enter_context(tc.tile_pool(name="psX", bufs=1, space="PSUM"))

        psAcc = psXp.tile([B, OUT], dt.float32)
        psT = psTp.tile([100, 2, OUT], dt.float32)

        xts = {}

        def phaseA_dma(k):
            lst = []
            for kc in range(KCH):
                xt = xtp.tile([128, CB], dt.bfloat16, tag="xt")
                nc.sync.dma_start(
                    out=xt[:],
                    in_=xbf2[k * CB:(k + 1) * CB, kc * 128:(kc + 1) * 128],
                    transpose=True,
                )
                lst.append(xt)
            xts[k] = lst

        def phaseA_compute(k):
            xt = xts.pop(k)
            d1s = []
            for mc in range(MCH):
                ps = psAB.tile([128, 1024], dt.float32, tag="psAB", name="psAB")
                for kc in range(KCH):
                    kp = 128 if kc < KCH - 1 else IN - 128 * (KCH - 1)
                    lhsT = sb["w1t"][0:kp, kc * HR + mc * 128:
                                     kc * HR + (mc + 1) * 128]
                    for nt in range(2):
                        nc.tensor.matmul(
                            ps[:, nt * 512:nt * 512 + 400], lhsT,
                            xt[kc][0:kp, nt * 400:(nt + 1) * 400],
                            start=(kc == 0), stop=(kc == KCH - 1),
                        )
                c1 = c1p.tile([128, B, TC + 1], dt.bfloat16, tag="c1")
                nc.scalar.copy(c1[:, :, 0:1], D1K[:, mc, :, :])
                nc.scalar.activation(
                    c1[:].rearrange("p (nt b) i -> p nt b i", nt=2)[:, :, :, 1:TC + 1],
                    ps[:].rearrange("p (nt x) -> p nt x", nt=2)[:, :, 0:400]
                        .rearrange("p nt (b t) -> p nt b t", t=TC),
                    Act.Identity, bias=sb["b1c"][:, mc:mc + 1], scale=1.0,
                )
                d1 = d1p.tile([128, B, TC + 1], dt.bfloat16, tag="d1")
                nc.vector.tensor_tensor_scan(
                    d1[:].rearrange("p b t -> p (b t)"),
                    sb["bz1c"][:, mc, :],
                    c1[:].rearrange("p b t -> p (b t)"),
                    0.0, op0=Alu.mult, op1=Alu.add,
                )
                nc.scalar.copy(D1K[:, mc, :, :], d1[:, :, TC:TC + 1])
                d1s.append(d1)
            slot = (k * TC) % VR
            for hh in range(2):
                ta = dap.tile([128, B, TC], dt.bfloat16, tag="ta")
                tb2 = dap.tile([128, B, TC], dt.bfloat16, tag="tb")
                nc.vector.tensor_tensor(
                    ta[:], d1s[hh][:, :, 1:], d1s[2 + hh][:, :, 1:], Alu.add)
                nc.vector.tensor_tensor(
                    tb2[:], d1s[4 + hh][:, :, 1:], d1s[6 + hh][:, :, 1:], Alu.add)
                nc.vector.tensor_tensor(
                    V[:, 0, hh, :, slot:slot + TC], ta[:], tb2[:], Alu.add)

        def phaseA_zero(k):
            slot = (k * TC) % VR
            nc.vector.memset(V[:, 0, :, :, slot:slot + TC], 0.0)

        def msteps(kk):
            for tau in range(kk * TC, (kk + 1) * TC):
                slot = tau % (2 * TC)
                sprev = (ZC[:] if tau == 0
                         else S[:, :, :, :, (tau - 1) % (2 * TC)])
                n = dap.tile([128, 2, 2, B], dt.bfloat16, tag="n")
                nc.vector.tensor_tensor(n[:], M[:], sprev, Alu.subtract)
                g = dap.tile([128, 2, 2, B], dt.bfloat16, tag="g")
                nc.vector.tensor_tensor(g[:], n[:], sb["abc"][:], Alu.mult)
                nc.vector.tensor_tensor(M[:], g[:], V[:, :, :, :, tau % VR],
                                        Alu.add)
                nc.vector.tensor_scalar(
                    S[:, :, :, :, slot], M[:], 1.0, None, op0=Alu.is_gt)

        def layer2(kk):
            base = (kk * TC) % (2 * TC)
            d2s = []
            for mc in range(MCH):
                ps = psAB.tile([128, 1024], dt.float32, tag="psAB", name="psAB")
                for kc in range(2):
                    for nt in range(2):
                        rhs = S[:, 0, kc, nt * 16:(nt + 1) * 16, base:base + TC]
                        nc.tensor.matmul(
                            ps[:, nt * 512:nt * 512 + 400],
                            sb["w2t"][:, kc * HR + mc * 128:
                                      kc * HR + (mc + 1) * 128],
                            rhs, start=(kc == 0), stop=(kc == 1),
                        )
                c2 = c2p.tile([128, B, TC + 1], dt.bfloat16, tag="c2")
                nc.scalar.copy(c2[:, :, 0:1], D2K[:, mc, :, :])
                nc.scalar.activation(
                    c2[:].rearrange("p (nt b) i -> p nt b i", nt=2)[:, :, :, 1:TC + 1],
                    ps[:].rearrange("p (nt x) -> p nt x", nt=2)[:, :, 0:400]
                        .rearrange("p nt (b t) -> p nt b t", t=TC),
                    Act.Identity, bias=sb["b2c"][:, mc:mc + 1], scale=1.0,
                )
                d2 = d2p.tile([128, B, TC + 1], dt.bfloat16, tag="d2")
                nc.vector.tensor_tensor_scan(
                    d2[:].rearrange("p b t -> p (b t)"),
                    sb["bz2"][:, mc, :],
                    c2[:].rearrange("p b t -> p (b t)"),
                    0.0, op0=Alu.mult, op1=Alu.add,
                )
                nc.scalar.copy(D2K[:, mc, :, :], d2[:, :, TC:TC + 1])
                d2s.append(d2)
            slot = ((kk + 2) * TC) % VR
            for hh in range(2):
                ta = dap.tile([128, B, TC], dt.bfloat16, tag="ta")
                tb2 = dap.tile([128, B, TC], dt.bfloat16, tag="tb")
                nc.vector.tensor_tensor(
                    ta[:], d2s[hh][:, :, 1:], d2s[2 + hh][:, :, 1:], Alu.add)
                nc.vector.tensor_tensor(
                    tb2[:], d2s[4 + hh][:, :, 1:], d2s[6 + hh][:, :, 1:], Alu.add)
                nc.vector.tensor_tensor(
                    V[:, 1, hh, :, slot:slot + TC], ta[:], tb2[:], Alu.add)

        def readout(kk):
            # output-time chunk kk; layer-2 spikes live at tau chunk kk+1
            base = ((kk + 2) * TC) % (2 * TC)
            ps3 = ps3p.tile([OUT, 1024], dt.float32, tag="ps3")
            for kc in range(2):
                for nt in range(2):
                    rhs = S[:, 1, kc, nt * 16:(nt + 1) * 16, base:base + TC]
                    nc.tensor.matmul(
                        ps3[:, nt * 512:nt * 512 + 400],
                        sb["wrt"][:, kc * OUT:(kc + 1) * OUT], rhs,
                        start=(kc == 0), stop=(kc == 1),
                    )
            c3 = rop.tile([OUT, B, TC + 1], dt.float32, tag="c3")
            nc.scalar.copy(c3[:, :, 0:1], MRK[:])
            nc.scalar.activation(
                c3[:].rearrange("p (nt b) i -> p nt b i", nt=2)[:, :, :, 1:TC + 1],
                ps3[:].rearrange("p (nt x) -> p nt x", nt=2)[:, :, 0:400]
                     .rearrange("p nt (b t) -> p nt b t", t=TC),
                Act.Identity, bias=sb["brc"][:], scale=1.0,
            )
            mrt = rop.tile([OUT, B, TC + 1], dt.float32, tag="mrt")
            nc.vector.tensor_tensor_scan(
                mrt[:].rearrange("p b t -> p (b t)"),
                sb["bzr"][:],
                c3[:].rearrange("p b t -> p (b t)"),
                0.0, op0=Alu.mult, op1=Alu.add,
            )
            nc.scalar.copy(MRK[:], mrt[:, :, TC:TC + 1])
            ex = rop.tile([OUT, TC, B], dt.float32, tag="ex")
            nc.scalar.activation(
                ex[:].rearrange("p t b -> p b t"),
                mrt[:, :, 1:TC + 1], Act.Exp,
            )
            # softmax + accumulate: 8 blocks of 100 t-major columns
            exf = ex[:].rearrange("p t b -> p (t b)")
            for l in range(8):
                pt = psT[:, l % 2, :]
                nc.tensor.transpose(pt, exf[:, l * 100:(l + 1) * 100],
                                    sb["idm"][:])
                rs = smp.tile([100, 1], dt.float32, tag="rs")
                nc.vector.tensor_reduce(rs[:], pt,
                                        axis=mybir.AxisListType.X, op=Alu.add)
                ri = smp.tile([100, 1], dt.float32, tag="ri")
                nc.vector.reciprocal(ri[:], rs[:])
                sm = smp.tile([100, OUT], dt.float32, tag="sm")
                nc.vector.tensor_scalar(sm[:], pt, ri[:], None, op0=Alu.mult)
                if kk == 0 and l == 0:
                    continue  # cols 0-99 are warmup (t<4)
                le = 8 if (kk == 0 and l == 1) else l
                nc.tensor.matmul(
                    psAcc[:],
                    sb["esel8"][:, le * B:(le + 1) * B],
                    sm[:],
                    start=(kk == 0 and l == 1),
                    stop=(kk == NCHUNK - 1 and l == 7),
                )

        # ---------- the pipelined chunk loop ----------
        for k in range(NCHUNK + 3):
            if k < NCHUNK:
                phaseA_dma(k)
            if k in (NCHUNK, NCHUNK + 1):
                phaseA_zero(k)
            if 1 <= k <= NCHUNK + 2:
                msteps(k - 1)
            if 1 <= k <= NCHUNK:
                layer2(k - 1)
            if k >= 3:
                readout(k - 3)
            if k < NCHUNK:
                phaseA_compute(k)

        accS = smp.tile([B, OUT], dt.float32, tag="acc")
        nc.scalar.copy(accS[:], psAcc[:])
        nc.scalar.dma_start(out=out[:], in_=accS[:])

    nc.compile()
    return nc


_NC_CACHE = {}


def _get_program(num_devices=NCORES):
    if num_devices not in _NC_CACHE:
        _NC_CACHE[num_devices] = _build_program(num_devices)
    return _NC_CACHE[num_devices]


def make_in_maps(x, consts):
    xs = np.ascontiguousarray(x.astype(np.float32).reshape(NCORES, ROWS, IN))
    return [{"x": xs[c], **consts} for c in range(NCORES)]


def kernel(x, W1, b1, tau_n1, tau_m1, W2, b2, tau_n2, tau_m2, Wr, br, tau_mr):
    from concourse.bass_utils import run_bass_kernel_spmd

    consts = _prep_constants(W1, b1, tau_n1, tau_m1, W2, b2, tau_n2, tau_m2,
                             Wr, br, tau_mr)
    nc = _get_program()
    in_maps = make_in_maps(np.asarray(x), consts)
    res = run_bass_kernel_spmd(nc, in_maps, list(range(NCORES)))
    outk = "outv"
    o = np.concatenate([res.results[c][outk] for c in range(NCORES)], axis=0)
    return o.astype(np.float32)


# revision 10
# speedup vs baseline: 3.0184x; 1.3242x over previous
"""Trainium2 Bass kernel for nn_DHSFNN_6038724018534 (v2, chunk-pipelined).

Two-layer dendritic-branch spiking net, data parallel over batch (8 cores x 32).

v2 structure: everything is chunked by 25 time steps and pipelined:
  cast:   x fp32 -> xbf2 bf16 DRAM, rows reordered (chunk, batch, t_local)
          so each chunk's x^T transpose is one contiguous [800,128] xbar DMA.
  chunk k: transpose-DMA -> mm1 (PE) -> c1 bias (ScalarE) -> dendrite scan
          (Pool engine) -> branch-sum adds (VectorE) -> V ring (3 chunks).
  The serial membrane/spike scan (msteps, VectorE) for chunk k-1 runs while
  chunk k's drive is produced; layer 2 (lagged 25 steps) and the softmax
  readout are also per-chunk, so PE/Pool/ScalarE/DVE all overlap.
"""

import numpy as np
import ml_dtypes

BF16 = ml_dtypes.bfloat16

B_FULL, T, IN, H, OUT, BR = 256, 250, 700, 256, 20, 4
NCORES = 8
B = B_FULL // NCORES          # 32 per core
ROWS = B * T                  # 8000
INP = 768                     # padded contraction dim
KCH = 6                       # k-chunks for matmul1 (last has 60 rows)
HR = H * BR                   # 1024
MCH = HR // 128               # 8
TC = 25                       # chunk length
NCHUNK = T // TC              # 10
LAG = 2 * TC
TAU = T + LAG                 # 275 merged-scan steps
VR = 3 * TC                   # V ring length (75)
CB = B * TC                   # 800 columns per chunk
CBB = B * (TC + 1)            # 832 with boundary col


def _sigmoid(v):
    return 1.0 / (1.0 + np.exp(-v.astype(np.float64)))


def _prep_constants(W1, b1, tau_n1, tau_m1, W2, b2, tau_n2, tau_m2, Wr, br, tau_mr):
    beta1 = _sigmoid(tau_n1)          # [H, BR]
    a1 = _sigmoid(tau_m1)             # [H]
    beta2 = _sigmoid(tau_n2)
    a2 = _sigmoid(tau_m2)
    ar = _sigmoid(tau_mr)             # [OUT]

    mj, mh = np.divmod(np.arange(HR), H)          # m = j*H + h
    orig = mh * BR + mj
    scale1 = (1.0 - beta1[mh, mj]) * (1.0 - a1[mh])
    scale2 = (1.0 - beta2[mh, mj]) * (1.0 - a2[mh])

    w1p = W1.astype(np.float64)[orig] * scale1[:, None]
    w2p = W2.astype(np.float64)[orig] * scale2[:, None]
    b1p = b1.astype(np.float64)[orig] * scale1
    b2p = b2.astype(np.float64)[orig] * scale2

    w1pad = np.zeros((HR, INP), np.float64)
    w1pad[:, :IN] = w1p
    w1t = np.ascontiguousarray(
        w1pad.T.reshape(KCH, 128, HR).transpose(1, 0, 2).reshape(128, KCH * HR)
    ).astype(BF16)
    w2t = np.ascontiguousarray(
        w2p.T.reshape(2, 128, HR).transpose(1, 0, 2).reshape(128, 2 * HR)
    ).astype(BF16)

    b1c = np.ascontiguousarray(b1p.reshape(MCH, 128).T).astype(np.float32)
    b2c = np.ascontiguousarray(b2p.reshape(MCH, 128).T).astype(np.float32)

    beta1_m = beta1[mh, mj].reshape(MCH, 128).T          # [128, MCH]
    beta2_m = beta2[mh, mj].reshape(MCH, 128).T

    def bz(beta_m):
        # [128, MCH, B*(TC+1)] with 0 at each (b, i==0) boundary col
        z = np.repeat(beta_m[:, :, None], B * (TC + 1), axis=2).reshape(
            128, MCH, B, TC + 1)
        z[:, :, :, 0] = 0.0
        return z.reshape(128, MCH, CBB).astype(BF16)

    bz1c = bz(beta1_m)
    bz2 = bz(beta2_m)

    abc = np.empty((128, 2, 2, B), np.float64)
    for hh in range(2):
        abc[:, 0, hh, :] = a1[hh * 128:(hh + 1) * 128, None]
        abc[:, 1, hh, :] = a2[hh * 128:(hh + 1) * 128, None]
    abc = abc.astype(BF16)

    wrp = Wr.astype(np.float64) * (1.0 - ar)[:, None]
    wrt = np.ascontiguousarray(
        wrp.T.reshape(2, 128, OUT).transpose(1, 0, 2).reshape(128, 2 * OUT)
    ).astype(BF16)
    brc = (br.astype(np.float64) * (1.0 - ar)).astype(np.float32).reshape(OUT, 1)

    # mr-scan coefficients: [20, B*(TC+1)], 0 at boundary cols
    bzr = np.repeat(ar[:, None], B * (TC + 1), axis=1).reshape(OUT, B, TC + 1)
    bzr[:, :, 0] = 0.0
    bzr = bzr.reshape(OUT, CBB).astype(np.float32)

    # esel8[p, l*B + b] = ((p + 4*l) % 32 == b), for 100-col softmax blocks
    esel8 = np.zeros((100, 9, B), np.float32)
    for l in range(8):
        for p in range(100):
            esel8[p, l, (p + 4 * l) % B] = 1.0
    # slot 8: like l=1 but rows p<28 zeroed (warmup mask for chunk 0, block 1)
    esel8[28:, 8, :] = esel8[28:, 1, :]
    esel8 = esel8.reshape(100, 9 * B)

    idm = np.eye(OUT, dtype=np.float32)

    return dict(w1t=w1t, w2t=w2t, b1c=b1c, b2c=b2c, bz1c=bz1c, bz2=bz2,
                abc=abc, wrt=wrt, brc=brc, bzr=bzr, esel8=esel8, idm=idm)


def _build_program(num_devices=NCORES):
    from contextlib import ExitStack
    import concourse.bacc as bacc
    import concourse.tile as tile
    from concourse import mybir

    dt = mybir.dt
    Alu = mybir.AluOpType
    Act = mybir.ActivationFunctionType

    nc = bacc.Bacc("TRN2", target_bir_lowering=False, debug=False,
                   num_devices=num_devices)

    x = nc.dram_tensor("x", [ROWS, IN], dt.float32, kind="ExternalInput").ap()
    cons = {}
    for name, shape, d in [
        ("w1t", [128, KCH * HR], dt.bfloat16),
        ("w2t", [128, 2 * HR], dt.bfloat16),
        ("b1c", [128, MCH], dt.float32),
        ("b2c", [128, MCH], dt.float32),
        ("bz1c", [128, MCH, CBB], dt.bfloat16),
        ("bz2", [128, MCH, CBB], dt.bfloat16),
        ("abc", [128, 2, 2, B], dt.bfloat16),
        ("wrt", [128, 2 * OUT], dt.bfloat16),
        ("brc", [OUT, 1], dt.float32),
        ("bzr", [OUT, CBB], dt.float32),
        ("esel8", [100, 9 * B], dt.float32),
        ("idm", [OUT, OUT], dt.float32),
    ]:
        cons[name] = nc.dram_tensor(name, shape, d, kind="ExternalInput").ap()
    out = nc.dram_tensor("outv", [B, OUT], dt.float32, kind="ExternalOutput").ap()

    xbf2 = nc.dram_tensor("xbf2", [NCHUNK * CB, INP], dt.bfloat16).ap()

    with tile.TileContext(nc) as tc, ExitStack() as ctx:
        cpool = ctx.enter_context(tc.tile_pool(name="consts", bufs=1))
        sb = {}
        for name in cons:
            ap = cons[name]
            t = cpool.tile(list(ap.shape), ap.dtype, name=f"sb_{name}",
                           tag=f"sb_{name}")
            nc.gpsimd.dma_start(out=t[:], in_=ap[:])
            sb[name] = t

        gpool = ctx.enter_context(tc.tile_pool(name="glob", bufs=1))
        V = gpool.tile([128, 2, 2, B, VR], dt.bfloat16)      # drive ring
        S = gpool.tile([128, 2, 2, B, 2 * TC], dt.bfloat16)  # spike ring
        M = gpool.tile([128, 2, 2, B], dt.bfloat16)
        ZC = gpool.tile([128, 2, 2, B], dt.bfloat16)
        D1K = gpool.tile([128, MCH, B, 1], dt.bfloat16)
        D2K = gpool.tile([128, MCH, B, 1], dt.bfloat16)
        MRK = gpool.tile([OUT, B, 1], dt.float32)

        nc.vector.memset(M[:], 0.0)
        nc.vector.memset(ZC[:], 0.0)
        nc.vector.memset(D1K[:], 0.0)
        nc.vector.memset(D2K[:], 0.0)
        nc.vector.memset(MRK[:], 0.0)
        nc.vector.memset(V[:, 1, :, :, 0:2 * TC], 0.0)  # layer-2 drive, tau<50

        # ---------- cast: x -> xbf2, rows reordered (k, b, tl) ----------
        castp = ctx.enter_context(tc.tile_pool(name="cast", bufs=3))
        xv = x.rearrange("(b t) c -> b t c", t=T)
        # out view: [k, b, tl, c] row = k*800 + b*25 + tl
        ov = xbf2.rearrange("(k b tl) c -> k b tl c", b=B, tl=TC)
        for h in range(2):
            for g in range(B // 2):
                tf = castp.tile([125, 2, IN], dt.float32, tag="castf")
                nc.sync.dma_start(
                    out=tf[:],
                    in_=xv[2 * g:2 * g + 2, 125 * h:125 * (h + 1), :]
                        .rearrange("b t c -> t b c"),
                )
                tb = castp.tile([125, 2, IN], dt.bfloat16, tag="castb")
                nc.scalar.copy(tb[:], tf[:])
                # SBUF [125(t), 2(b), c] -> DRAM, one DMA per chunk row
                for kl in range(5):
                    nc.sync.dma_start(
                        out=ov[5 * h + kl, 2 * g:2 * g + 2, :, 0:IN]
                            .rearrange("b tl c -> tl b c"),
                        in_=tb[kl * TC:(kl + 1) * TC, :, :],
                    )

        # ---------- pools for the chunk pipeline ----------
        xtp = ctx.enter_context(tc.tile_pool(name="xt", bufs=13))
        c1p = ctx.enter_context(tc.tile_pool(name="c1", bufs=3))
        c2p = ctx.enter_context(tc.tile_pool(name="c2", bufs=3))
        d1p = ctx.enter_context(tc.tile_pool(name="d1", bufs=1))
        d2p = ctx.enter_context(tc.tile_pool(name="d2", bufs=1))
        dap = ctx.enter_context(tc.tile_pool(name="da", bufs=4))
        rop = ctx.enter_context(tc.tile_pool(name="ro", bufs=2))
        smp = ctx.enter_context(tc.tile_pool(name="sm", bufs=4))
        psAB = ctx.enter_context(tc.tile_pool(name="psAB", bufs=2, space="PSUM"))
        ps3p = ctx.enter_context(tc.tile_pool(name="ps3", bufs=1, space="PSUM"))
        psTp = ctx.enter_context(tc.tile_pool(name="psT", bufs=1, space="PSUM"))
        psXp = ctx.enter_context(tc.tile_pool(name="psX", bufs=1, space="PSUM"))

        psAcc = psXp.tile([B, OUT], dt.float32)
        psT = psTp.tile([100, 2, OUT], dt.float32)

        xts = {}

        def phaseA_dma(k):
            lst = []
            for kc in range(KCH):
                xt = xtp.tile([128, CB], dt.bfloat16, tag="xt")
                nc.sync.dma_start(
                    out=xt[:],
                    in_=xbf2[k * CB:(k + 1) * CB, kc * 128:(kc + 1) * 128],
                    transpose=True,
                )
                lst.append(xt)
            xts[k] = lst

        def phaseA_compute(k):
            xt = xts.pop(k)
            d1T = d1p.tile([128, MCH, B, TC + 1], dt.bfloat16, tag="d1T")
            for mc in range(MCH):
                ps = psAB.tile([128, 1024], dt.float32, tag="psAB", name="psAB")
                for kc in range(KCH):
                    kp = 128 if kc < KCH - 1 else IN - 128 * (KCH - 1)
                    lhsT = sb["w1t"][0:kp, kc * HR + mc * 128:
                                     kc * HR + (mc + 1) * 128]
                    for nt in range(2):
                        nc.tensor.matmul(
                            ps[:, nt * 512:nt * 512 + 400], lhsT,
                            xt[kc][0:kp, nt * 400:(nt + 1) * 400],
                            start=(kc == 0), stop=(kc == KCH - 1),
                        )
                c1 = c1p.tile([128, B, TC + 1], dt.bfloat16, tag="c1")
                nc.scalar.copy(c1[:, :, 0:1], D1K[:, mc, :, :])
                nc.scalar.activation(
                    c1[:].rearrange("p (nt b) i -> p nt b i", nt=2)[:, :, :, 1:TC + 1],
                    ps[:].rearrange("p (nt x) -> p nt x", nt=2)[:, :, 0:400]
                        .rearrange("p nt (b t) -> p nt b t", t=TC),
                    Act.Identity, bias=sb["b1c"][:, mc:mc + 1], scale=1.0,
                )
                nc.vector.tensor_tensor_scan(
                    d1T[:, mc].rearrange("p b t -> p (b t)"),
                    sb["bz1c"][:, mc, :],
                    c1[:].rearrange("p b t -> p (b t)"),
                    0.0, op0=Alu.mult, op1=Alu.add,
                )
            nc.scalar.copy(D1K[:], d1T[:, :, :, TC:TC + 1])
            slot = (k * TC) % VR
            Dv = d1T[:].rearrange("p (j h) b i -> p j h b i", h=2)
            ta = dap.tile([128, 2, B, TC], dt.bfloat16, tag="ta")
            tb2 = dap.tile([128, 2, B, TC], dt.bfloat16, tag="tb")
            nc.vector.tensor_tensor(
                ta[:], Dv[:, 0, :, :, 1:], Dv[:, 1, :, :, 1:], Alu.add)
            nc.vector.tensor_tensor(
                tb2[:], Dv[:, 2, :, :, 1:], Dv[:, 3, :, :, 1:], Alu.add)
            nc.vector.tensor_tensor(
                V[:, 0, :, :, slot:slot + TC], ta[:], tb2[:], Alu.add)

        def phaseA_zero(k):
            slot = (k * TC) % VR
            nc.vector.memset(V[:, 0, :, :, slot:slot + TC], 0.0)

        def msteps(kk):
            for tau in range(kk * TC, (kk + 1) * TC):
                slot = tau % (2 * TC)
                sprev = (ZC[:] if tau == 0
                         else S[:, :, :, :, (tau - 1) % (2 * TC)])
                n = dap.tile([128, 2, 2, B], dt.bfloat16, tag="n")
                nc.vector.tensor_tensor(n[:], M[:], sprev, Alu.subtract)
                g = dap.tile([128, 2, 2, B], dt.bfloat16, tag="g")
                nc.vector.tensor_tensor(g[:], n[:], sb["abc"][:], Alu.mult)
                nc.vector.tensor_tensor(M[:], g[:], V[:, :, :, :, tau % VR],
                                        Alu.add)
                nc.vector.tensor_scalar(
                    S[:, :, :, :, slot], M[:], 1.0, None, op0=Alu.is_gt)

        def layer2(kk):
            base = (kk * TC) % (2 * TC)
            d2T = d2p.tile([128, MCH, B, TC + 1], dt.bfloat16, tag="d2T")
            for mc in range(MCH):
                ps = psAB.tile([128, 1024], dt.float32, tag="psAB", name="psAB")
                for kc in range(2):
                    for nt in range(2):
                        rhs = S[:, 0, kc, nt * 16:(nt + 1) * 16, base:base + TC]
                        nc.tensor.matmul(
                            ps[:, nt * 512:nt * 512 + 400],
                            sb["w2t"][:, kc * HR + mc * 128:
                                      kc * HR + (mc + 1) * 128],
                            rhs, start=(kc == 0), stop=(kc == 1),
                        )
                c2 = c2p.tile([128, B, TC + 1], dt.bfloat16, tag="c2")
                nc.scalar.copy(c2[:, :, 0:1], D2K[:, mc, :, :])
                nc.scalar.activation(
                    c2[:].rearrange("p (nt b) i -> p nt b i", nt=2)[:, :, :, 1:TC + 1],
                    ps[:].rearrange("p (nt x) -> p nt x", nt=2)[:, :, 0:400]
                        .rearrange("p nt (b t) -> p nt b t", t=TC),
                    Act.Identity, bias=sb["b2c"][:, mc:mc + 1], scale=1.0,
                )
                nc.vector.tensor_tensor_scan(
                    d2T[:, mc].rearrange("p b t -> p (b t)"),
                    sb["bz2"][:, mc, :],
                    c2[:].rearrange("p b t -> p (b t)"),
                    0.0, op0=Alu.mult, op1=Alu.add,
                )
            nc.scalar.copy(D2K[:], d2T[:, :, :, TC:TC + 1])
            slot = ((kk + 2) * TC) % VR
            Dv = d2T[:].rearrange("p (j h) b i -> p j h b i", h=2)
            ta = dap.tile([128, 2, B, TC], dt.bfloat16, tag="ta")
            tb2 = dap.tile([128, 2, B, TC], dt.bfloat16, tag="tb")
            nc.vector.tensor_tensor(
                ta[:], Dv[:, 0, :, :, 1:], Dv[:, 1, :, :, 1:], Alu.add)
            nc.vector.tensor_tensor(
                tb2[:], Dv[:, 2, :, :, 1:], Dv[:, 3, :, :, 1:], Alu.add)
            nc.vector.tensor_tensor(
                V[:, 1, :, :, slot:slot + TC], ta[:], tb2[:], Alu.add)

        def readout(kk):
            # output-time chunk kk; layer-2 spikes live at tau chunk kk+1
            base = ((kk + 2) * TC) % (2 * TC)
            ps3 = ps3p.tile([OUT, 1024], dt.float32, tag="ps3")
            for kc in range(2):
                for nt in range(2):
                    rhs = S[:, 1, kc, nt * 16:(nt + 1) * 16, base:base + TC]
                    nc.tensor.matmul(
                        ps3[:, nt * 512:nt * 512 + 400],
                        sb["wrt"][:, kc * OUT:(kc + 1) * OUT], rhs,
                        start=(kc == 0), stop=(kc == 1),
                    )
            c3 = rop.tile([OUT, B, TC + 1], dt.float32, tag="c3")
            nc.scalar.copy(c3[:, :, 0:1], MRK[:])
            nc.scalar.activation(
                c3[:].rearrange("p (nt b) i -> p nt b i", nt=2)[:, :, :, 1:TC + 1],
                ps3[:].rearrange("p (nt x) -> p nt x", nt=2)[:, :, 0:400]
                     .rearrange("p nt (b t) -> p nt b t", t=TC),
                Act.Identity, bias=sb["brc"][:], scale=1.0,
            )
            mrt = rop.tile([OUT, B, TC + 1], dt.float32, tag="mrt")
            nc.vector.tensor_tensor_scan(
                mrt[:].rearrange("p b t -> p (b t)"),
                sb["bzr"][:],
                c3[:].rearrange("p b t -> p (b t)"),
                0.0, op0=Alu.mult, op1=Alu.add,
            )
            nc.scalar.copy(MRK[:], mrt[:, :, TC:TC + 1])
            ex = rop.tile([OUT, TC, B], dt.float32, tag="ex")
            nc.scalar.activation(
                ex[:].rearrange("p t b -> p b t"),
                mrt[:, :, 1:TC + 1], Act.Exp,
            )
            # softmax + accumulate: 8 blocks of 100 t-major columns
            exf = ex[:].rearrange("p t b -> p (t b)")
            for l in range(8):
                pt = psT[:, l % 2, :]
                nc.tensor.transpose(pt, exf[:, l * 100:(l + 1) * 100],
                                    sb["idm"][:])
                rs = smp.tile([100, 1], dt.float32, tag="rs")
                nc.vector.tensor_reduce(rs[:], pt,
                                        axis=mybir.AxisListType.X, op=Alu.add)
                ri = smp.tile([100, 1], dt.float32, tag="ri")
                nc.vector.reciprocal(ri[:], rs[:])
                sm = smp.tile([100, OUT], dt.float32, tag="sm")
                nc.vector.tensor_scalar(sm[:], pt, ri[:], None, op0=Alu.mult)
                if kk == 0 and l == 0:
                    continue  # cols 0-99 are warmup (t<4)
                le = 8 if (kk == 0 and l == 1) else l
                nc.tensor.matmul(
                    psAcc[:],
                    sb["esel8"][:, le * B:(le + 1) * B],
                    sm[:],
                    start=(kk == 0 and l == 1),
                    stop=(kk == NCHUNK - 1 and l == 7),
                )

        # ---------- the pipelined chunk loop ----------
        phaseA_dma(0)
        for k in range(NCHUNK + 3):
            if k + 1 < NCHUNK:
                phaseA_dma(k + 1)
            if k in (NCHUNK, NCHUNK + 1):
                phaseA_zero(k)
            if 1 <= k <= NCHUNK + 2:
                msteps(k - 1)
            if k < NCHUNK:
                phaseA_compute(k)
            if 1 <= k <= NCHUNK:
                layer2(k - 1)
            if k >= 3:
                readout(k - 3)

        accS = smp.tile([B, OUT], dt.float32, tag="acc")
        nc.scalar.copy(accS[:], psAcc[:])
        nc.scalar.dma_start(out=out[:], in_=accS[:])

    nc.compile()
    return nc


_NC_CACHE = {}


def _get_program(num_devices=NCORES):
    if num_devices not in _NC_CACHE:
        _NC_CACHE[num_devices] = _build_program(num_devices)
    return _NC_CACHE[num_devices]


def make_in_maps(x, consts):
    xs = np.ascontiguousarray(x.astype(np.float32).reshape(NCORES, ROWS, IN))
    return [{"x": xs[c], **consts} for c in range(NCORES)]


def kernel(x, W1, b1, tau_n1, tau_m1, W2, b2, tau_n2, tau_m2, Wr, br, tau_mr):
    from concourse.bass_utils import run_bass_kernel_spmd

    consts = _prep_constants(W1, b1, tau_n1, tau_m1, W2, b2, tau_n2, tau_m2,
                             Wr, br, tau_mr)
    nc = _get_program()
    in_maps = make_in_maps(np.asarray(x), consts)
    res = run_bass_kernel_spmd(nc, in_maps, list(range(NCORES)))
    outk = "outv"
    o = np.concatenate([res.results[c][outk] for c in range(NCORES)], axis=0)
    return o.astype(np.float32)
